# revision 68
# baseline (speedup 1.0000x reference)
"""CantorAttention Trainium2 kernel — banded formulation, qt-major schedule.

Problem (hardcoded): B=2, S=2048, DIM=512, H=8 heads, D=64, K=64 routes.
  qkv = x @ w_qkv + b_qkv ; per-head sparse attention over routes[q, :] ;
  out = attn_out @ w_out + b_out.

Sharding (8 cores): batch x head-pairs. Core i handles batch i//4 and heads
(2*(i%4), 2*(i%4)+1).

Key idea: routes come from k-nearest Cantor coordinates. Sorting queries and
keys by Cantor coordinate (host-side permutation, free) makes the 0/1 route
mask C banded: ~46 of 256 [128x128] tiles are nonzero. The schedule is
derived at runtime from the actual `routes`, so the kernel stays correct for
arbitrary routes (it just degrades toward dense).

qt-major: for each 128-query tile, its 2-4 key tiles' scores land in one
psum tile -> one exp (ScalarE) -> one mask multiply -> 2-4 PV matmuls
accumulating [V|1]^T pm (ones column = softmax denominator). Score/exp/mask
chains are fused into phase 1's emission (the scalar engine runs QKV-copies
and exps back to back), while all PV matmuls are deferred into one dense
TensorE stream (masked probs persist in SBUF), followed by the output
projection. Output partials are bf16, summed + b_out on host.
"""

import numpy as np
import ml_dtypes

import concourse.bass as bass
import concourse.bacc as bacc
import concourse.mybir as mybir
import concourse.tile as tile
from concourse.bass_utils import run_bass_kernel_spmd
from concourse.masks import make_identity

BF16 = mybir.dt.bfloat16
F32 = mybir.dt.float32
NPBF16 = ml_dtypes.bfloat16

B = 2
S = 2048
DIM = 512
H = 8
D = 64
KR = 64
SCALE = 0.125

P = 128
NT = S // P       # 16 seq tiles
NC4 = DIM // P    # 4 contraction chunks

_CACHE = {}


def _cantor_perm():
    x = np.arange(S, dtype=np.float32) / max(1, S - 1)
    x = np.clip(x, 1e-6, 1.0 - 1e-6)
    val = np.zeros_like(x)
    factor = 0.5
    for _ in range(8):
        xs = x * 3.0
        digit = np.floor(xs).astype(np.int64)
        x = xs - digit.astype(np.float32)
        val = val + (digit == 2).astype(np.float32) * factor
        factor *= 0.5
    coords = np.clip(val, 0.0, 1.0)
    return np.lexsort((np.arange(S), coords))


def _schedule(routes):
    """kt-major banded schedule from the actual routes input.

    Returns (perm, sched, ct_packed):
      sched: tuple of entries (kt, qlo, n) with 1<=n<=4 — key tile kt scores
        against query tiles qlo..qlo+n-1 in one wide matmul (gap-filled, so
        zero ct blocks are harmless).
      ct_packed: [128, 128*sum(n)] bf16, C^T blocks in entry order.
    """
    perm = _cantor_perm()
    C = np.zeros((S, S), np.float32)
    np.add.at(C, (np.arange(S)[:, None], np.asarray(routes)), 1.0)
    Cp = C[perm][:, perm]
    occ = Cp.reshape(NT, P, NT, P).any(axis=(1, 3))  # [qt, kt]

    sched = []
    blocks = []
    for kt in range(NT):
        qts = np.nonzero(occ[:, kt])[0]
        if len(qts) == 0:
            continue
        lo, hi = int(qts.min()), int(qts.max()) + 1
        start = lo
        while start < hi:
            n = min(4, hi - start)
            sched.append((kt, start, n))
            blk = Cp[start * P:(start + n) * P, kt * P:(kt + 1) * P].T
            blocks.append(np.ascontiguousarray(blk))
            start += n
    ct_packed = np.concatenate(blocks, axis=1).astype(NPBF16)
    return perm, tuple(sched), ct_packed


def build_nc(sched):
    key = ("nc", sched)
    if key in _CACHE:
        return _CACHE[key]
    nc = bacc.Bacc(
        "TRN2",
        target_bir_lowering=False,
        debug=False,
        num_devices=8,
    )

    nU = len(sched)
    # ct column offset per entry, and 4 DMA chunks cut at entry boundaries
    u_off = []
    off = 0
    for (kt, qlo, n) in sched:
        u_off.append(off)
        off += n * P
    ctw = off
    ct_cuts = [0]
    for i in range(1, 4):
        tgt = ctw * i // 4
        ct_cuts.append(min((o for o in u_off if o >= tgt), default=ctw))
    ct_cuts.append(ctw)
    u_chunk = []
    for o in u_off:
        ci = max(i for i in range(4) if ct_cuts[i] <= o)
        u_chunk.append((ci, o - ct_cuts[ci]))
    # per qt: first/last entry covering it (for psum start/stop flags)
    first_u = {}
    last_u = {}
    for u, (kt, qlo, n) in enumerate(sched):
        for qt in range(qlo, qlo + n):
            if qt not in first_u:
                first_u[qt] = u
            last_u[qt] = u
    assert all(qt in first_u for qt in range(NT)), "query tile with no routes"
    # fused emission: entry ready once its kh/qh column ranges are copied
    ready_qc = []
    for (kt, qlo, n) in sched:
        ready_qc.append(max(kt // 4, (qlo + n - 1) // 4))
    deferred = nU <= 48  # defer PV only if pm tiles fit comfortably in SBUF

    w_d = nc.dram_tensor("w", [P, 3 * NC4 * P], BF16, kind="ExternalInput").ap()
    b_d = nc.dram_tensor("b", [P, 3], F32, kind="ExternalInput").ap()
    xt_d = {}
    for g in range(2):
        xt_d[g] = nc.dram_tensor(
            f"xt_{g}", [P, NC4 * 1024], BF16, kind="ExternalInput").ap()
    ct_d = nc.dram_tensor("ct", [P, ctw], BF16, kind="ExternalInput").ap()
    wo_d = nc.dram_tensor("wo", [P, DIM], BF16, kind="ExternalInput").ap()
    out_d = nc.dram_tensor("out", [P, 4 * S], BF16, kind="ExternalOutput").ap()

    with tile.TileContext(nc) as tc:
        with tc.tile_pool(name="persist", bufs=1) as pp:
            # identity + warm-up assets first: PE ramps p-state while DMAs run
            ident = pp.tile([P, P], BF16, tag="ident")
            make_identity(nc, ident[:])
            zwarm = pp.tile([P, 512], BF16, tag="zwarm", name="zwarm")
            nc.vector.memset(zwarm[:], 0.0)  # keep first in the vector queue

            # inputs split across all three DMA queues (~93GB/s each):
            # the first phase-1 matmul needs w + xt_g0 = 1.4MB, so halve
            # every big tensor across sync/scalar
            w_all = pp.tile([P, 3 * NC4 * P], BF16, tag="wall", name="w_all")
            nc.sync.dma_start(out=w_all[:, 0:768], in_=w_d[:, 0:768])
            nc.scalar.dma_start(out=w_all[:, 768:1536], in_=w_d[:, 768:1536])
            w_sb = {name: w_all[:, i * NC4 * P:(i + 1) * NC4 * P]
                    for i, name in enumerate(("q", "k", "v"))}
            b_all = pp.tile([P, 3], F32, tag="ball", name="b_all")
            b_sb = {name: b_all[:, i:i + 1]
                    for i, name in enumerate(("q", "k", "v"))}
            xt_sb = {}
            # xt_g0 is on the phase-1 critical path: 3-way split
            t = pp.tile([P, NC4 * 1024], BF16, tag="xt0", name="xt0")
            nc.sync.dma_start(out=t[:, 0:1536], in_=xt_d[0][:, 0:1536])
            nc.scalar.dma_start(out=t[:, 1536:3072], in_=xt_d[0][:, 1536:3072])
            nc.gpsimd.dma_start(out=t[:, 3072:4096], in_=xt_d[0][:, 3072:4096])
            xt_sb[0] = t
            t = pp.tile([P, NC4 * 1024], BF16, tag="xt1", name="xt1")
            nc.sync.dma_start(out=t[:, 0:2048], in_=xt_d[1][:, 0:2048])
            nc.scalar.dma_start(out=t[:, 2048:4096], in_=xt_d[1][:, 2048:4096])
            xt_sb[1] = t
            nc.gpsimd.dma_start(out=b_all[:], in_=b_d[:, :])
            # ct on the gpsimd queue, chunked to pace mask consumption
            ct_sb = []
            for i in range(4):
                w_i = ct_cuts[i + 1] - ct_cuts[i]
                t = pp.tile([P, w_i], BF16, tag=f"ct{i}", name=f"ct{i}")
                ct_sb.append(t)
            nc.gpsimd.dma_start(
                out=ct_sb[0][:], in_=ct_d[:, ct_cuts[0]:ct_cuts[1]])
            wo_sb = pp.tile([P, DIM], BF16, tag="wo", name="wo_sb")

            # q/k both heads stacked on partitions (h0 rows 0-63, h1 64-127)
            qh = pp.tile([P, S], BF16, tag="qh", name="qh")
            kh = pp.tile([P, S], BF16, tag="kh", name="kh")
            vt = pp.tile([P, S], BF16, tag="vt", name="vt")
            v2 = {}
            for h in range(2):
                for kt in range(NT):
                    v2[(h, kt)] = pp.tile([P, D + 1], BF16, tag=f"v2_{h}_{kt}",
                                          name=f"v2_{h}_{kt}")
            ot_sb = pp.tile([P, S], BF16, tag="ot", name="ot_sb")
            den_t = [pp.tile([1, S], BF16, tag=f"den{h}", name=f"den{h}")
                     for h in range(2)]
            ones64 = pp.tile([1, D], BF16, tag="ones64", name="ones64")
            nc.vector.memset(ones64[:], 1.0)
            # per-head reciprocal denominators broadcast to 64 partitions
            r2r = [pp.tile([D, S], F32, tag=f"r2r{h}", name=f"r2r{h}")
                   for h in range(2)]
            ot1_sb = pp.tile([D, S], BF16, tag="ot1", name="ot1_sb")
            on_sb = pp.tile([P, S], BF16, tag="on", name="on_sb")
            # ones columns on vector (gpsimd queue stays clear for ident+DMA)
            for h in range(2):
                for kt in range(NT):
                    nc.vector.memset(v2[(h, kt)][:, D:D + 1], 1.0)
            for i in range(1, 4):
                nc.gpsimd.dma_start(
                    out=ct_sb[i][:], in_=ct_d[:, ct_cuts[i]:ct_cuts[i + 1]])
            nc.gpsimd.dma_start(out=wo_sb[:], in_=wo_d[:, :])

            pm_t = {}
            if deferred:
                for h in range(2):
                    for u in range(nU):
                        pm_t[(h, u)] = pp.tile(
                            [P, 512], BF16, tag=f"pm{h}_{u}", name=f"pm{h}_{u}")

            # --- phase 0: PE p-state warm-up during the input DMA window ---
            with tc.tile_pool(name="wp", bufs=2, space="PSUM") as wpp:
                for _ in range(6):
                    w_ps = wpp.tile([P, 512], F32, tag="warm", name="warm")
                    nc.tensor.matmul(
                        w_ps[:], lhsT=ident[:], rhs=zwarm[:],
                        start=True, stop=True, skip_group_check=True,
                    )

            ot_ps = {}

            def emit_sem(h, u, sp, ppool, mask_eng):
                kt, qlo, n = sched[u]
                W = n * P
                ci, lo = u_chunk[u]
                hr = slice(h * D, (h + 1) * D)
                s_ps = sp.tile([P, 512], F32, tag="s", name="s_ps")
                nc.tensor.matmul(
                    s_ps[:, 0:W],
                    lhsT=kh[hr, kt * P:(kt + 1) * P],
                    rhs=qh[hr, qlo * P:qlo * P + W],
                    start=True, stop=True,
                )
                p_sb = ppool.tile([P, 512], BF16, tag="p", name="p_sb")
                nc.scalar.activation(
                    p_sb[:, 0:W], s_ps[:, 0:W],
                    mybir.ActivationFunctionType.Exp,
                )
                pm_sb = pm_t[(h, u)] if deferred else \
                    ppool.tile([P, 512], BF16, tag="pm", name="pm_sb")
                mask_eng.tensor_tensor(
                    out=pm_sb[:, 0:W], in0=p_sb[:, 0:W],
                    in1=ct_sb[ci][:, lo:lo + W],
                    op=mybir.AluOpType.mult,
                )
                return pm_sb

            def emit_pv(h, u, pm_sb):
                kt, qlo, n = sched[u]
                for j in range(n):
                    qt = qlo + j
                    # interleave qt -> bank qt%4: concurrently open psum
                    # accumulation groups always sit in distinct banks
                    pc = (qt % 4) * 512 + (qt // 4) * P
                    nc.tensor.matmul(
                        ot_ps[h][0:D + 1, pc:pc + P],
                        lhsT=v2[(h, kt)][:],
                        rhs=pm_sb[:, j * P:(j + 1) * P],
                        start=(u == first_u[qt]),
                        stop=(u == last_u[qt]),
                    )

            def emit_drain0():
                # h0 ot drain — runs on scalar/vector during h1's PVs
                nc.scalar.copy(out=ot_sb[0:D, 0:1024], in_=ot_ps[0][0:D, 0:1024])
                nc.vector.tensor_copy(
                    out=ot_sb[0:D, 1024:S], in_=ot_ps[0][0:D, 1024:S])

            # --- phase 1 fused with score/exp/mask chains ---
            # scores for units of qc-1 are emitted during qc's block so the
            # scalar queue runs [copies(qc) | exps(qc-1)] without starvation
            with tc.tile_pool(name="sp", bufs=2, space="PSUM") as sp, \
                 tc.tile_pool(name="ppool", bufs=3) as ppool:
                with tc.tile_pool(name="ps1", bufs=2, space="PSUM") as p1:
                    for qc in range(5):
                        if qc < 4:
                            g, half = qc // 2, qc % 2
                            ps = {}
                            for name in ("q", "k", "v"):
                                ps[name] = p1.tile([P, 512], F32,
                                                   tag=f"p1{name}",
                                                   name=f"p1{name}")
                            for c in range(NC4):
                                for name in ("q", "k", "v"):
                                    nc.tensor.matmul(
                                        ps[name][:],
                                        lhsT=w_sb[name][:, c * P:(c + 1) * P],
                                        rhs=xt_sb[g][:, c * 1024 + half * 512:
                                                     c * 1024 + (half + 1) * 512],
                                        start=(c == 0),
                                        stop=(c == NC4 - 1),
                                    )
                            qs = slice(qc * 512, (qc + 1) * 512)
                            nc.scalar.activation(
                                qh[:, qs], ps["q"][:],
                                mybir.ActivationFunctionType.Identity,
                                bias=b_sb["q"][:])
                            nc.vector.tensor_tensor(
                                out=kh[:, qs], in0=ps["k"][:],
                                in1=b_sb["k"][:].to_broadcast([P, 512]),
                                op=mybir.AluOpType.add)
                            nc.vector.tensor_tensor(
                                out=vt[:, qs], in0=ps["v"][:],
                                in1=b_sb["v"][:].to_broadcast([P, 512]),
                                op=mybir.AluOpType.add)
                        if qc > 0 and deferred:
                            for h in range(2):
                                eng = nc.vector if h == 0 else nc.gpsimd
                                for u in range(nU):
                                    if ready_qc[u] == qc - 1:
                                        emit_sem(h, u, sp, ppool, eng)

                # --- phase 1b: V into [key, d] layout via PE transpose ---
                with tc.tile_pool(name="tp", bufs=2, space="PSUM") as tpp:
                    for kt in range(NT):
                        ks = slice(kt * P, (kt + 1) * P)
                        tp = tpp.tile([P, P], BF16, tag="tp", name="tp")
                        nc.tensor.transpose(out=tp[:], in_=vt[:, ks],
                                            identity=ident[:])
                        nc.vector.tensor_copy(out=v2[(0, kt)][:, 0:D],
                                              in_=tp[:, 0:D])
                        nc.vector.tensor_copy(out=v2[(1, kt)][:, 0:D],
                                              in_=tp[:, D:P])

            # --- phase 2: dense deferred PV streams, both heads ---
            # op1 outer so op0's banks free up for the r2 pool while h1's
            # psum is still being read by the fused normalize
            if deferred:
                with tc.tile_pool(name="op1", bufs=1, space="PSUM") as op1p:
                    ot_ps[1] = op1p.tile([P, S], F32, tag="otps1", name="otps1")
                    with tc.tile_pool(name="op0", bufs=1, space="PSUM") as op0p:
                        ot_ps[0] = op0p.tile([P, S], F32, tag="otps0",
                                             name="otps0")
                        for u in range(nU):
                            emit_pv(0, u, pm_t[(0, u)])
                        # h0 den + drain, runs during h1's PV stream below
                        # (vector only: scalar still has exp backlog here)
                        nc.vector.tensor_copy(out=den_t[0][0:1, 0:1024],
                                              in_=ot_ps[0][D:D + 1, 0:1024])
                        nc.vector.tensor_copy(out=den_t[0][0:1, 1024:S],
                                              in_=ot_ps[0][D:D + 1, 1024:S])
                        emit_drain0()
                    # op0 closed: banks free for the den-broadcast pool
                    with tc.tile_pool(name="rp", bufs=2, space="PSUM") as rp:
                        for u in range(nU):
                            emit_pv(1, u, pm_t[(1, u)])
                        for qc in range(4):
                            qs = slice(qc * 512, (qc + 1) * 512)
                            eng = nc.scalar.copy if qc % 2 == 0 else \
                                nc.vector.tensor_copy
                            eng(out=den_t[1][0:1, qs],
                                in_=ot_ps[1][D:D + 1, qs])
                        # warm fillers hold the PE p-state through the
                        # den->r2->recip chain so the finals run fast
                        with tc.tile_pool(name="wp2", bufs=2,
                                          space="PSUM") as wp2:
                            for qc in range(4):
                                qs = slice(qc * 512, (qc + 1) * 512)
                                w_ps = wp2.tile([P, 512], F32, tag="warm2",
                                                name="warm2")
                                nc.tensor.matmul(
                                    w_ps[:], lhsT=ident[:], rhs=zwarm[:],
                                    start=True, stop=True,
                                    skip_group_check=True)
                                for h in range(2):
                                    r2_ps = rp.tile([D, 512], F32, tag="r2",
                                                    name="r2_ps")
                                    nc.tensor.matmul(
                                        r2_ps[:], lhsT=ones64[:],
                                        rhs=den_t[h][:, qs],
                                        start=True, stop=True)
                                    nc.vector.reciprocal_approx_fast(
                                        out=r2r[h][:, qs], in_=r2_ps[:])
                        for qc in range(4):
                            qs = slice(qc * 512, (qc + 1) * 512)
                            if qc % 2 == 0:
                                nc.vector.tensor_tensor(
                                    out=on_sb[D:P, qs], in0=ot_ps[1][0:D, qs],
                                    in1=r2r[1][:, qs], op=mybir.AluOpType.mult)
                            else:
                                nc.scalar.copy(out=ot1_sb[:, qs],
                                               in_=ot_ps[1][0:D, qs])
                                nc.gpsimd.tensor_tensor(
                                    out=on_sb[D:P, qs], in0=ot1_sb[:, qs],
                                    in1=r2r[1][:, qs], op=mybir.AluOpType.mult)
                            nc.gpsimd.tensor_tensor(
                                out=on_sb[0:D, qs], in0=ot_sb[0:D, qs],
                                in1=r2r[0][:, qs], op=mybir.AluOpType.mult)
            else:
                # fallback for non-banded routes: sequential, no deferral
                with tc.tile_pool(name="sp2", bufs=2, space="PSUM") as sp2, \
                     tc.tile_pool(name="pw2", bufs=4) as pw2:
                    for h in range(2):
                        with tc.tile_pool(name=f"op{h}", bufs=1,
                                          space="PSUM") as opp:
                            ot_ps[h] = opp.tile([P, S], F32, tag="otps",
                                                name="otps")
                            for u in range(nU):
                                pm = emit_sem(h, u, sp2, pw2, nc.vector)
                                emit_pv(h, u, pm)
                            nc.scalar.copy(out=den_t[h][0:1, :],
                                           in_=ot_ps[h][D:D + 1, :])
                            dst = ot_sb[0:D, :] if h == 0 else ot1_sb[:, :]
                            nc.scalar.copy(out=dst, in_=ot_ps[h][0:D, :])
                with tc.tile_pool(name="rpf", bufs=2, space="PSUM") as rp:
                    for qc in range(4):
                        qs = slice(qc * 512, (qc + 1) * 512)
                        for h in range(2):
                            hs = slice(h * D, (h + 1) * D)
                            r2_ps = rp.tile([D, 512], F32, tag="r2",
                                            name="r2_ps")
                            nc.tensor.matmul(
                                r2_ps[:], lhsT=ones64[:],
                                rhs=den_t[h][:, qs],
                                start=True, stop=True,
                            )
                            nc.vector.reciprocal_approx_fast(
                                out=r2r[h][:, qs], in_=r2_ps[:])
                            src = ot_sb[0:D, qs] if h == 0 else ot1_sb[:, qs]
                            nc.vector.tensor_tensor(
                                out=on_sb[hs, qs], in0=src,
                                in1=r2r[h][:, qs], op=mybir.AluOpType.mult)

            # --- phase 3: output projection (transposed out); each half is
            # DMA'd as soon as its copy lands, on rotating queues ---
            with tc.tile_pool(name="fp", bufs=3, space="PSUM") as fp, \
                 tc.tile_pool(name="fsb", bufs=4) as fsb:
                qi = 0
                for oc in range(4):
                    for g in range(2):
                        f_ps = fp.tile([P, 1024], F32, tag="f", name="f_ps")
                        for qq in range(2):
                            cs = slice(g * 1024 + qq * 512,
                                       g * 1024 + (qq + 1) * 512)
                            nc.tensor.matmul(
                                f_ps[:, qq * 512:(qq + 1) * 512],
                                lhsT=wo_sb[:, oc * P:(oc + 1) * P],
                                rhs=on_sb[:, cs],
                                start=True, stop=True,
                            )
                        f_sb = fsb.tile([P, 1024], BF16, tag="fsb",
                                        name="f_sb")
                        if g == 0:
                            nc.vector.tensor_copy(out=f_sb[:], in_=f_ps[:])
                        else:
                            nc.scalar.copy(out=f_sb[:], in_=f_ps[:])
                        dma_eng = (nc.sync, nc.scalar, nc.gpsimd)[qi % 3]
                        qi += 1
                        dma_eng.dma_start(
                            out=out_d[:, oc * S + g * 1024:
                                      oc * S + (g + 1) * 1024],
                            in_=f_sb[:])

    nc.compile()
    _CACHE[key] = nc
    return nc


def _pack_w(a):
    # [512, 128] -> [128, 4*128] with row chunk c at cols c*128..
    return np.ascontiguousarray(
        a.reshape(NC4, P, P).transpose(1, 0, 2).reshape(P, NC4 * P))


def make_in_maps(x, routes, w_qkv, b_qkv, w_out):
    x = np.asarray(x, np.float32)
    w_qkv = np.asarray(w_qkv, np.float32)
    b_qkv = np.asarray(b_qkv, np.float32)
    w_out = np.asarray(w_out, np.float32)

    perm, sched, ct_packed = _schedule(routes)

    xts = {}
    for b in range(B):
        xT = np.ascontiguousarray(x[b][perm].T)  # [512, 2048] permuted
        for g in range(2):
            # [128, 4*1024]: contraction chunk c at cols c*1024..
            xts[(b, g)] = np.ascontiguousarray(
                xT[:, g * 1024:(g + 1) * 1024]
                .reshape(NC4, P, 1024).transpose(1, 0, 2)
                .reshape(P, NC4 * 1024)).astype(NPBF16)

    in_maps = []
    for core in range(8):
        b = core // 4
        hp = core % 4
        col = hp * P
        w_parts = [
            _pack_w(w_qkv[:, col:col + P] * SCALE),
            _pack_w(w_qkv[:, DIM + col:DIM + col + P]),
            _pack_w(w_qkv[:, 2 * DIM + col:2 * DIM + col + P]),
        ]
        b_parts = np.stack([
            b_qkv[col:col + P] * SCALE,
            b_qkv[DIM + col:DIM + col + P],
            b_qkv[2 * DIM + col:2 * DIM + col + P],
        ], axis=1)
        m = dict(
            w=np.concatenate(w_parts, axis=1).astype(NPBF16),
            b=np.ascontiguousarray(b_parts).astype(np.float32),
            ct=ct_packed,
            wo=np.ascontiguousarray(w_out[col:col + P, :]).astype(NPBF16),
            xt_0=xts[(b, 0)],
            xt_1=xts[(b, 1)],
        )
        in_maps.append(m)
    return in_maps, perm, sched


_COLMAP = np.array([(q // P % 4) * 512 + (q // P // 4) * P + q % P
                    for q in range(S)])


def unpack_out(arr, perm):
    """[128, 4*2048] bf16 core output -> [2048, 512] f32 in original order."""
    outT = np.zeros((DIM, S), np.float32)
    a = np.asarray(arr, np.float32)
    for oc in range(4):
        outT[oc * P:(oc + 1) * P, :] = a[:, oc * S:(oc + 1) * S]
    out = np.zeros((S, DIM), np.float32)
    out[perm] = outT[:, _COLMAP].T
    return out


def run(inputs, trace=False, trace_cores=None):
    in_maps, perm, sched = make_in_maps(
        inputs["x"], inputs["routes"], inputs["w_qkv"], inputs["b_qkv"],
        inputs["w_out"],
    )
    nc = build_nc(sched)
    res = run_bass_kernel_spmd(
        nc, in_maps, list(range(8)), trace=trace, trace_cores=trace_cores,
    )
    b_out = np.asarray(inputs["b_out"], np.float32)
    final = np.zeros((B, S, DIM), np.float32)
    for core in range(8):
        final[core // 4] += unpack_out(res.results[core]["out"], perm)
    final += b_out[None, None, :]
    return final, res


def kernel(**inputs):
    final, _ = run(inputs, trace=False)
    return final
